# revision 17
# baseline (speedup 1.0000x reference)
"""Trainium2 Bass kernel for a dense transformer block (B=4, T=2048, E=1024,
H=16, D_FF=4096, causal attention, pre-LN, fp32 reference).

Sharding: 8 cores = (batch b in 0..3) x (parity c in 0..1). Core (b, c) owns
query blocks {c, c+2, ..., c+14} (128 rows each) of batch b. Each core
computes the full K/V projections for its batch (cheap duplication) and the
attention + FFN rows only for its own query blocks. Zero collectives.

Layout: all activations on device are kept transposed ([feature, token]) so
every matmul contraction lands on the partition axis with no on-device
transposes:
  hT = LN1(xT);  qT = Wq^T.h^T via (kxm=Wq, kxn=hT_own);  kT likewise;
  v  = h.Wv via (kxm=hT_full, kxn=Wv)  (natural layout, used as AV lhsT);
  scores^T[k,q] = kT_h.T @ qT_h (contraction d=64, two heads row-packed);
  exp on ScalarE (no max subtraction: |scores|*0.125 <~ 6, exact softmax
  identity); causal masking is multiplicative {0,1} tiles passed as data;
  AV with a ones-column appended to v gives unnormalized attn^T and the
  softmax denominator in one accumulation; normalize by DVE reciprocal.
Matmul dtypes: float32r (TF32-ish, full PE rate at free-dim>=256, ~1.5e-4)
for all projections/FFN; bf16 inside attention (free dims of 128).
"""
import sys

for _p in ("/opt/trn_rl_repo", "/opt/pypackages"):
    if _p not in sys.path:
        sys.path.append(_p)

import numpy as np
import ml_dtypes

import concourse.bass as bass
import concourse.mybir as mybir
import concourse.tile as tile
from concourse.bass_utils import run_bass_kernel_spmd
from concourse.kernels.tile_matmul import matmul_tile_kernel

B, T, E = 4, 2048, 1024
H, HS = 16, 64
D_FF = 4 * E
EPS = 1e-5
P = 128
NB = T // P          # 16 global query/key blocks
NJ = NB // 2         # 8 query blocks per core
TO = NJ * P          # 1024 own tokens per core

F32 = mybir.dt.float32
F32R = mybir.dt.float32r
BF16 = mybir.dt.bfloat16
ALU = mybir.AluOpType
ACT = mybir.ActivationFunctionType


# --------------------------------------------------------------------------
# post-pass: this walrus build rejects any instruction with >1 sync wait;
# hoist excess waits onto single-wait NoOps on the same (in-order) engine
# queue immediately before the instruction.
def _split_waits(nc, max_waits=1):
    n = 0
    for f in nc.m.functions:
        for blk in f.blocks:
            out, changed = [], False
            for inst in blk.instructions:
                si = inst.sync_info
                if si is not None and len(si.on_wait) > max_waits:
                    waits = list(si.on_wait)
                    for w in waits[:-max_waits]:
                        out.append(mybir.InstNoOp(
                            name=f"I-wf{nc.next_id()}", engine=inst.engine,
                            ins=[], outs=[],
                            sync_info=mybir.SyncInfo(on_wait=[w], on_update=[]),
                        ))
                    inst.sync_info = mybir.SyncInfo(
                        on_wait=waits[-max_waits:], on_update=list(si.on_update))
                    changed = True
                    n += 1
                out.append(inst)
            if changed:
                blk.instructions = out
    return n


def _bcast_from_dram(ap_1d_free, parts):
    """AP replicating a [1, ...] DRAM row across `parts` partitions."""
    return bass.AP(tensor=ap_1d_free.tensor, offset=ap_1d_free.offset,
                   ap=[[0, parts]] + list(ap_1d_free.ap[1:]))


def _ln_T(tc, name, eps_t, stat_dram, x_dram, out_dram, g_sb, b_sb, t_len):
    """LayerNorm over the feature axis in transposed layout.

    x_dram [E, t_len] f32(-compatible); out_dram [E, t_len] float32r.
    g_sb/b_sb: [128, E//128] f32 SBUF tiles (per-partition gain/bias chunks).
    stat_dram [2, t_len] f32 scratch for the mean/rstd broadcast roundtrip.
    Feature axis = partitions across EC chunks; per-token stats via
    ones-matmul partition reduction (f32r, free dim 512).
    """
    nc = tc.nc
    EC = E // P
    NT = t_len // 512
    with (
        tc.tile_pool(name=f"{name}_w", bufs=1) as work,
        tc.tile_pool(name=f"{name}_w2", bufs=2) as work2,
        tc.tile_pool(name=f"{name}_s", bufs=1) as stats,
        tc.tile_pool(name=f"{name}_ps", bufs=1, space="PSUM") as pspool,
    ):
        ones_f = stats.tile([P, 1], F32, tag="ln_ones_f")
        nc.vector.memset(ones_f[:], 1.0)
        ones_r = stats.tile([P, 1], F32R, tag="ln_ones")
        nc.vector.tensor_copy(ones_r[:], ones_f[:])

        x_r = x_dram.bitcast(F32R).rearrange("(c p) t -> p c t", p=P)

        xsb = work.tile([P, EC, t_len], F32R, tag="ln_x")
        nc.sync.dma_start(out=xsb[:], in_=x_r)

        sum_ps = [pspool.tile([1, 512], F32, name=f"ln_sum{i}", tag=f"ln_sum{i}") for i in range(NT)]
        sq_ps = [pspool.tile([1, 512], F32, name=f"ln_sq{i}", tag=f"ln_sq{i}") for i in range(NT)]
        for c in range(EC):
            sq = work2.tile([P, t_len], F32R, tag="ln_sqt")
            nc.vector.tensor_mul(sq[:], xsb[:, c, :], xsb[:, c, :])
            for i in range(NT):
                nc.tensor.matmul(sum_ps[i][:], ones_r[:], xsb[:, c, i * 512:(i + 1) * 512],
                                 start=(c == 0), stop=(c == EC - 1))
                nc.tensor.matmul(sq_ps[i][:], ones_r[:], sq[:, i * 512:(i + 1) * 512],
                                 start=(c == 0), stop=(c == EC - 1))

        mean = stats.tile([1, t_len], F32, tag="ln_mean")
        rstd = stats.tile([1, t_len], F32, tag="ln_rstd")
        msq = stats.tile([1, 512], F32, tag="ln_msq")
        for i in range(NT):
            sl = slice(i * 512, (i + 1) * 512)
            nc.vector.tensor_scalar_mul(mean[:, sl], sum_ps[i][:], 1.0 / E)
            nc.vector.tensor_mul(msq[:], mean[:, sl], mean[:, sl])
            # var = sumsq/E - mean^2 ; rstd = 1/sqrt(var + eps)
            nc.vector.tensor_scalar(rstd[:, sl], sq_ps[i][:], 1.0 / E, None, op0=ALU.mult)
            nc.vector.tensor_sub(rstd[:, sl], rstd[:, sl], msq[:])
            nc.scalar.activation(rstd[:, sl], rstd[:, sl], ACT.Sqrt, bias=eps_t[:])
            nc.vector.reciprocal(rstd[:, sl], rstd[:, sl])

        # roundtrip through DRAM to replicate per-token stats across partitions
        nc.sync.dma_start(out=stat_dram[0:1, :], in_=mean[:])
        nc.sync.dma_start(out=stat_dram[1:2, :], in_=rstd[:])
        mrep = work.tile([P, t_len], F32, tag="ln_mrep")
        rrep = work.tile([P, t_len], F32, tag="ln_rrep")
        nc.sync.dma_start(out=mrep[:], in_=_bcast_from_dram(stat_dram[0:1, :], P))
        nc.sync.dma_start(out=rrep[:], in_=_bcast_from_dram(stat_dram[1:2, :], P))

        out_r = out_dram.rearrange("(c p) t -> p c t", p=P)
        for c in range(EC):
            tmp = work2.tile([P, t_len], F32, tag="ln_tmp")
            hout = work2.tile([P, t_len], F32R, tag="ln_out")
            nc.vector.tensor_sub(tmp[:], xsb[:, c, :], mrep[:])
            nc.vector.tensor_mul(tmp[:], tmp[:], rrep[:])
            nc.vector.tensor_scalar(hout[:], tmp[:], g_sb[:, c:c + 1], b_sb[:, c:c + 1],
                                    op0=ALU.mult, op1=ALU.add)
            nc.sync.dma_start(out=out_r[:, c, :], in_=hout[:])


def _vec_chunks(consts, nc, dram_vec, name, n):
    """Load a [n] dram vector as [128, n/128] per-partition chunks."""
    t = consts.tile([P, n // P], F32, tag=name)
    nc.sync.dma_start(out=t[:], in_=dram_vec.rearrange("(c p) -> p c", p=P))
    return t


def _bias_m(bias_sb):
    """post_mxn_tile_fn adding a per-m-row bias (bias on partition axis)."""
    def fn(nc, sbuf, md, _):
        for s in range(md.m_subtiles):
            gidx = (md.m_tile_idx * md.m_tile) // P + s
            nc.vector.tensor_scalar_add(sbuf[:, s, :], sbuf[:, s, :],
                                        bias_sb[:, gidx:gidx + 1])
    return fn


def _bias_m_relu(bias_sb):
    def fn(nc, sbuf, md, _):
        for s in range(md.m_subtiles):
            gidx = (md.m_tile_idx * md.m_tile) // P + s
            nc.vector.tensor_scalar(sbuf[:, s, :], sbuf[:, s, :],
                                    bias_sb[:, gidx:gidx + 1], 0.0,
                                    op0=ALU.add, op1=ALU.max)
    return fn


def _build_program():
    nc = bass.Bass()

    xT = nc.declare_dram_parameter("xT", [E, T], F32, isOutput=False)
    xTo = nc.declare_dram_parameter("xTown", [E, TO], F32, isOutput=False)
    mask = nc.declare_dram_parameter("mask", [2, P, P], BF16, isOutput=False)
    Wq = nc.declare_dram_parameter("Wq", [E, E], F32R, isOutput=False)
    Wk = nc.declare_dram_parameter("Wk", [E, E], F32R, isOutput=False)
    Wv = nc.declare_dram_parameter("Wv", [E, E], F32R, isOutput=False)
    Wp = nc.declare_dram_parameter("Wp", [E, E], F32R, isOutput=False)
    W1 = nc.declare_dram_parameter("W1", [E, D_FF], F32R, isOutput=False)
    W2 = nc.declare_dram_parameter("W2", [D_FF, E], F32R, isOutput=False)
    bq = nc.declare_dram_parameter("bq", [E], F32, isOutput=False)
    bk = nc.declare_dram_parameter("bk", [E], F32, isOutput=False)
    bv = nc.declare_dram_parameter("bv", [E], F32, isOutput=False)
    bp = nc.declare_dram_parameter("bp", [E], F32, isOutput=False)
    b1 = nc.declare_dram_parameter("b1", [D_FF], F32, isOutput=False)
    b2 = nc.declare_dram_parameter("b2", [E], F32, isOutput=False)
    g1 = nc.declare_dram_parameter("g1", [E], F32, isOutput=False)
    be1 = nc.declare_dram_parameter("be1", [E], F32, isOutput=False)
    g2 = nc.declare_dram_parameter("g2", [E], F32, isOutput=False)
    be2 = nc.declare_dram_parameter("be2", [E], F32, isOutput=False)
    outT = nc.declare_dram_parameter("outT", [E, TO], F32, isOutput=True)

    hT = nc.dram_tensor("hT", [E, T], F32R)
    hTo = nc.dram_tensor("hTo", [E, TO], F32R)
    qT = nc.dram_tensor("qT", [E, TO], BF16)
    kT = nc.dram_tensor("kT", [E, T], BF16)
    vN = nc.dram_tensor("vN", [T, E], BF16)
    attnT = nc.dram_tensor("attnT", [E, TO], F32R)
    x2T = nc.dram_tensor("x2T", [E, TO], F32)
    h2T = nc.dram_tensor("h2T", [E, TO], F32R)
    uT = nc.dram_tensor("uT", [D_FF, TO], F32R)
    lnst1 = nc.dram_tensor("lnst1", [2, T], F32)
    lnst2 = nc.dram_tensor("lnst2", [2, TO], F32)
    lnst3 = nc.dram_tensor("lnst3", [2, TO], F32)

    with tile.TileContext(nc) as tc:
        with (
            tc.tile_pool(name="consts", bufs=1) as consts,
        ):
            g1_sb = _vec_chunks(consts, nc, g1[:], "g1", E)
            be1_sb = _vec_chunks(consts, nc, be1[:], "be1", E)
            g2_sb = _vec_chunks(consts, nc, g2[:], "g2", E)
            be2_sb = _vec_chunks(consts, nc, be2[:], "be2", E)
            bq_sb = _vec_chunks(consts, nc, bq[:], "bq", E)
            bk_sb = _vec_chunks(consts, nc, bk[:], "bk", E)
            bp_sb = _vec_chunks(consts, nc, bp[:], "bp", E)
            b1_sb = _vec_chunks(consts, nc, b1[:], "b1", D_FF)
            b2_sb = _vec_chunks(consts, nc, b2[:], "b2", E)
            bv_sb = consts.tile([1, E], F32, tag="bv")
            nc.sync.dma_start(out=bv_sb[:], in_=bv[None, :])
            eps_t = consts.tile([1, 1], F32, tag="ln_eps")
            nc.vector.memset(eps_t[:], EPS)

            _ln_T(tc, "ln1f", eps_t, lnst1[:], xT[:], hT[:], g1_sb, be1_sb, T)
            _ln_T(tc, "ln1o", eps_t, lnst2[:], xTo[:], hTo[:], g1_sb, be1_sb, TO)

            # --- projections (f32r) ---
            def mtk(kxm, kxn, mxn, m_bufs, n_bufs, **kw):
                with (
                    tc.tile_pool(name="mtk_kxm", bufs=m_bufs) as pm,
                    tc.tile_pool(name="mtk_kxn", bufs=n_bufs) as pn,
                    tc.tile_pool(name="mtk_acc", bufs=3) as pa,
                ):
                    matmul_tile_kernel(tc, kxm, kxn, mxn,
                                       kxm_pool=pm, kxn_pool=pn,
                                       accum_pool=pa, **kw)

            mtk(Wq[:], hTo[:], qT[:], 3, 5, post_mxn_tile_fn=_bias_m(bq_sb))
            mtk(Wk[:], hT[:], kT[:], 5, 3, post_mxn_tile_fn=_bias_m(bk_sb))
            mtk(hT[:], Wv[:], vN[:], 3, 5)  # bv added at v_aug build

            # --- attention (bf16) ---
            with (
                tc.tile_pool(name="at_in", bufs=1) as at_in,
                tc.tile_pool(name="at_exp", bufs=10) as at_exp,
                tc.tile_pool(name="at_out", bufs=4) as at_out,
                tc.tile_pool(name="at_sps", bufs=2, space="PSUM") as at_sps,
                tc.tile_pool(name="at_rep", bufs=2, space="PSUM") as at_rep,
                tc.tile_pool(name="at_psa", bufs=2, space="PSUM") as at_psa,
                tc.tile_pool(name="at_r", bufs=8) as at_r,
            ):
                q_sb = at_in.tile([P, NJ, TO], BF16, tag="q_sb")
                nc.sync.dma_start(out=q_sb[:], in_=qT.rearrange("(a p) t -> p a t", p=P))
                k_sb = at_in.tile([P, NJ, T], BF16, tag="k_sb")
                nc.sync.dma_start(out=k_sb[:], in_=kT.rearrange("(a p) t -> p a t", p=P))
                m_sb = at_in.tile([P, 2, P], BF16, tag="m_sb")
                nc.sync.dma_start(out=m_sb[:], in_=mask.rearrange("m p q -> p m q"))

                v_aug = at_in.tile([P, NB, H, HS + 1], BF16, tag="v_aug")
                bv_rep = at_in.tile([P, E], F32, tag="bv_rep")
                nc.sync.dma_start(out=bv_rep[:], in_=_bcast_from_dram(bv[None, :], P))
                with tc.tile_pool(name="at_vraw", bufs=2) as at_vraw:
                    nc.vector.memset(v_aug[:, :, :, HS:HS + 1], 1.0)
                    bv3 = bv_rep.rearrange("p (h d) -> p h d", h=H)
                    v_r = vN.rearrange("(a p) e -> p a e", p=P)
                    for kb in range(NB):
                        v_raw = at_vraw.tile([P, E], BF16, tag="v_raw")
                        nc.sync.dma_start(out=v_raw[:], in_=v_r[:, kb, :])
                        nc.vector.tensor_add(
                            v_aug[:, kb, :, 0:HS],
                            v_raw.rearrange("p (h d) -> p h d", h=H),
                            bv3)

                ones_hf = at_in.tile([1, HS], F32, tag="ones_hf")
                nc.vector.memset(ones_hf[:], 1.0)
                ones_h = at_in.tile([1, HS], mybir.dt.float16, tag="ones_h")
                nc.vector.tensor_copy(ones_h[:], ones_hf[:])

                aT_w = attnT.rearrange("(a p) t -> p a t", p=P)
                for j in range(NJ):
                    K = 2 * (j + 1)
                    qsl = slice(j * P, (j + 1) * P)
                    for hp in range(NJ):
                        pa = [at_psa.tile([HS + 1, P], F32, name=f"av_ps{h2}", tag=f"av_ps{h2}")
                              for h2 in range(2)]
                        for kb in range(K):
                            ksl = slice(kb * P, (kb + 1) * P)
                            for h2 in range(2):
                                psl = slice(h2 * HS, (h2 + 1) * HS)
                                ps = at_sps.tile([P, P], F32, tag="score_ps")
                                nc.tensor.matmul(ps[:], k_sb[psl, hp, ksl],
                                                 q_sb[psl, hp, qsl],
                                                 start=True, stop=True,
                                                 tile_position=(h2 * HS, 0))
                                ex = at_exp.tile([P, P], BF16, tag="exp_sb")
                                nc.scalar.activation(ex[:], ps[:], ACT.Exp,
                                                     scale=float(HS ** -0.5))
                                if kb >= K - 2:
                                    nc.vector.tensor_mul(ex[:], ex[:],
                                                         m_sb[:, kb - (K - 2), :])
                                h = 2 * hp + h2
                                nc.tensor.matmul(pa[h2][:], v_aug[:, kb, h, :],
                                                 ex[:],
                                                 start=(kb == 0), stop=(kb == K - 1))
                        # evict: softmax denominators -> fp16 outer-product
                        # replication -> single normalize mul
                        araw = at_out.tile([P, P], F32, tag="araw")
                        rep = at_rep.tile([P, P], F32, name="rep_ps", tag="rep_ps")
                        att = at_out.tile([P, P], F32R, tag="att_sb")
                        for h2 in range(2):
                            asl = slice(h2 * HS, (h2 + 1) * HS)
                            nc.vector.tensor_copy(araw[asl, :], pa[h2][0:HS, :])
                            rc = at_r.tile([1, P], mybir.dt.float16, tag="recip")
                            with nc.allow_low_precision(reason="softmax denom fp16 for fast replication"):
                                nc.vector.reciprocal(rc[:], pa[h2][HS:HS + 1, :])
                            nc.tensor.matmul(rep[asl, :], ones_h[:], rc[:],
                                             start=True, stop=True,
                                             tile_position=(0, h2 * HS))
                        nc.vector.tensor_mul(att[:], araw[:], rep[:])
                        nc.sync.dma_start(out=aT_w[:, hp, qsl], in_=att[:])

            # --- attention proj + residual ---
            mtk(Wp[:], attnT[:], x2T[:], 5, 5,
                accumulate_ap=xTo[:], post_mxn_tile_fn=_bias_m(bp_sb))

            # --- FFN ---
            _ln_T(tc, "ln2", eps_t, lnst3[:], x2T[:], h2T[:], g2_sb, be2_sb, TO)
            mtk(W1[:], h2T[:], uT[:], 3, 5,
                post_mxn_tile_fn=_bias_m_relu(b1_sb))
            # FFN2 split into two K-halves so cache pools stay small;
            # second half DMA-accumulates onto outT.
            KH = D_FF // 2
            mtk(W2[0:KH, :], uT[0:KH, :], outT[:], 5, 5,
                accumulate_ap=x2T[:], post_mxn_tile_fn=_bias_m(b2_sb))
            mtk(W2[KH:D_FF, :], uT[KH:D_FF, :], outT[:], 5, 5,
                mxn_accum_op=ALU.add)

    _split_waits(nc)
    return nc


_NC_CACHE = None


def _get_program():
    global _NC_CACHE
    if _NC_CACHE is None:
        _NC_CACHE = _build_program()
    return _NC_CACHE


def _make_masks():
    k = np.arange(P)[:, None]
    q = np.arange(P)[None, :]
    tri = (k <= q).astype(np.float32)
    m = np.zeros((2, 2, P, P), np.float32)
    m[0, 0] = tri          # c=0: kblock K-2 is diagonal
    m[0, 1] = 0.0          # c=0: kblock K-1 fully future
    m[1, 0] = 1.0          # c=1: kblock K-2 fully past
    m[1, 1] = tri          # c=1: kblock K-1 is diagonal
    return m.astype(ml_dtypes.bfloat16)


def make_in_maps(x, ln1_g, ln1_b, Wq, bq, Wk, bk, Wv, bv, Wp, bp,
                 ln2_g, ln2_b, W1, b1, W2, b2):
    x = np.asarray(x, np.float32)
    shared = {
        "Wq": np.ascontiguousarray(Wq, np.float32),
        "Wk": np.ascontiguousarray(Wk, np.float32),
        "Wv": np.ascontiguousarray(Wv, np.float32),
        "Wp": np.ascontiguousarray(Wp, np.float32),
        "W1": np.ascontiguousarray(W1, np.float32),
        "W2": np.ascontiguousarray(W2, np.float32),
        "bq": np.asarray(bq, np.float32), "bk": np.asarray(bk, np.float32),
        "bv": np.asarray(bv, np.float32), "bp": np.asarray(bp, np.float32),
        "b1": np.asarray(b1, np.float32), "b2": np.asarray(b2, np.float32),
        "g1": np.asarray(ln1_g, np.float32), "be1": np.asarray(ln1_b, np.float32),
        "g2": np.asarray(ln2_g, np.float32), "be2": np.asarray(ln2_b, np.float32),
    }
    masks = _make_masks()
    in_maps = []
    for core in range(8):
        b, c = divmod(core, 2)
        xb = x[b]                                   # [T, E]
        xT = np.ascontiguousarray(xb.T)             # [E, T]
        own = xb.reshape(NB, P, E)[c::2].reshape(TO, E)
        xTown = np.ascontiguousarray(own.T)         # [E, TO]
        in_maps.append({**shared, "xT": xT, "xTown": xTown,
                        "mask": np.ascontiguousarray(masks[c])})
    return in_maps


def assemble_output(results, dtype=np.float32):
    out = np.empty((B, T, E), dtype)
    for core, res in enumerate(results):
        b, c = divmod(core, 2)
        x3 = np.asarray(res["outT"]).T              # [TO, E]
        out[b].reshape(NB, P, E)[c::2] = x3.reshape(NJ, P, E)
    return out


def kernel(**inputs):
    nc = _get_program()
    in_maps = make_in_maps(**inputs)
    res = run_bass_kernel_spmd(nc, in_maps, list(range(8)))
    return assemble_output(res.results)
